# revision 3
# baseline (speedup 1.0000x reference)
"""Trainium2 Bass kernel for batched Hadamard transform.

Computes out = (x_re + i*x_im) @ H where H is the 4096x4096 Walsh-Hadamard
unitary (real, entries +-1/64).  Since H is real, out_re = x_re @ H and
out_im = x_im @ H independently.

Algorithm: H_4096 = H_64 (x) H_64 (Kronecker), so each 4096-row, viewed as a
64x64 matrix V, transforms as  H64 . V . H64  -- a 32x FLOP reduction vs the
dense matmul.  On the tensor engine this is implemented as
    MM(contract i) -> PE-transpose -> MM(contract j) -> PE-transpose
with a single constant stationary blockdiag(H64, H64) for both MMs (full
128-partition contraction, two rows packed per 64-column group).

Sharding: data-parallel over the batch dim (8 batches -> 8 NeuronCores),
x_re/x_im rows of one batch processed as 2*512 independent rows per core.
"""

import os
import re
import numpy as np

from concourse import bass, tile
import concourse.mybir as mybir
from concourse.bass_utils import run_bass_kernel_spmd
from concourse.tile import TileContext
from concourse.tile_sem_assignment import tick_to_sem


def _drain_and_barrier_split(self, tick_clock, wait_clock):
    # The stock kernel-tail drain carries one sem-wait per active proc on a
    # single instruction; this walrus build rejects >2 sync waits per
    # instruction ("Too many sync wait commands").  Emit one wait_ge per
    # proc instead, then a bare drain.
    gc = tick_clock.global_clock
    ticks = [int(v) for v in re.findall(r"\d+", repr(gc))]
    for proc, sem in sorted(self.sems.allocated().items()):
        if proc < len(ticks) and ticks[proc] > 0:
            self.nc.sync.wait_ge(sem, tick_to_sem(ticks[proc], proc))
    self.nc.sync.drain()
    self.nc.all_engine_barrier()
    assert self.sems is not None
    popped = self.nc._tile_sem_poison_stack.pop()
    assert popped is self._sem_poison
    self.nc.clear_and_free_semaphores(list(self.sems.allocated().values()))
    self.nc.all_engine_barrier()


TileContext._drain_and_barrier = _drain_and_barrier_split

_MAX_WAITS = 1


def _split_excess_waits(nc):
    """This walrus build rejects instructions with >2 sync-wait commands.
    Move excess waits onto same-engine NoOps inserted just before the
    instruction (engines execute their queue in order, so the sync semantics
    are preserved)."""
    n_split = 0
    for fn in nc.m.functions:
        for bb in fn.blocks:
            insts = list(bb.instructions)
            out = []
            for inst in insts:
                si = inst.sync_info
                waits = list(si.on_wait) if si and si.on_wait else []
                if len(waits) > _MAX_WAITS:
                    extra = waits[: len(waits) - _MAX_WAITS]
                    keep = waits[len(waits) - _MAX_WAITS :]
                    for ci in range(0, len(extra), _MAX_WAITS):
                        chunk = extra[ci : ci + _MAX_WAITS]
                        n_split += 1
                        nop = mybir.InstNoOp(
                            name=f"waitnop-{n_split}-{inst.name}",
                            engine=inst.engine,
                            sync_info=mybir.SyncInfo(
                                on_wait=list(chunk), on_update=[]
                            ),
                        )
                        out.append(nop)
                    inst.sync_info = mybir.SyncInfo(
                        on_wait=list(keep), on_update=list(si.on_update)
                    )
                out.append(inst)
            if len(out) != len(insts):
                bb.instructions = out
    return n_split

B, M, N = 8, 512, 4096
NCORES = 8
G = 32          # row-groups per tensor; 16 rows per group
F32 = mybir.dt.float32
F32R = mybir.dt.float32r

# toggles (env for experimentation)
# fp32r: PE streams fp32 at 1 cyc/row (vs 4) for ap>=256 and 1.5 (vs 2) in
# transpose mode.  Producers must emit "rounded" fp32r, so the PSUM->SBUF
# copies write fp32r-typed tiles; MM1 stays plain fp32 (fed directly by DMA).
USE_F32R = os.environ.get("HAD_F32R", "0") == "1"
BUFS = int(os.environ.get("HAD_BUFS", "3"))


def _hadamard(n: int) -> np.ndarray:
    h = np.array([[1.0]], dtype=np.float64)
    while h.shape[0] < n:
        h = np.block([[h, h], [h, -h]])
    return h


def _host_constants():
    h64 = (_hadamard(64) / 8.0).astype(np.float32)      # normalized: H64 @ H64 = I
    hh = np.zeros((128, 128), dtype=np.float32)
    hh[:64, :64] = h64
    hh[64:, 64:] = h64
    ident = np.eye(128, dtype=np.float32)
    return hh, ident


def _build():
    DT = F32R if USE_F32R else F32
    nc = bass.Bass()
    xre = nc.dram_tensor("x_re", [M, N], F32, kind="ExternalInput")
    xim = nc.dram_tensor("x_im", [M, N], F32, kind="ExternalInput")
    hh = nc.dram_tensor("hh", [128, 128], F32, kind="ExternalInput")
    ident = nc.dram_tensor("ident", [128, 128], F32, kind="ExternalInput")
    ore = nc.dram_tensor("o_re", [M, N], F32, kind="ExternalOutput")
    oim = nc.dram_tensor("o_im", [M, N], F32, kind="ExternalOutput")

    with tile.TileContext(nc) as tc:
        with (
            tc.tile_pool(name="const", bufs=1) as cpool,
            tc.tile_pool(name="a", bufs=BUFS) as apool,
            tc.tile_pool(name="b1", bufs=BUFS) as b1pool,
            tc.tile_pool(name="b2", bufs=BUFS) as b2pool,
            tc.tile_pool(name="b3", bufs=BUFS) as b3pool,
            tc.tile_pool(name="c", bufs=BUFS) as cpool2,
            tc.tile_pool(name="ps1", bufs=2, space="PSUM") as ps1pool,
            tc.tile_pool(name="psf1", bufs=2, space="PSUM") as psf1pool,
            tc.tile_pool(name="ps2", bufs=2, space="PSUM") as ps2pool,
            tc.tile_pool(name="psf2", bufs=2, space="PSUM") as psf2pool,
        ):
            hh_sb = cpool.tile([128, 128], F32)
            id_sb = cpool.tile([128, 128], F32)
            nc.sync.dma_start(hh_sb[:], hh[:])
            nc.sync.dma_start(id_sb[:], ident[:])
            if USE_F32R:
                hh_r = cpool.tile([128, 128], F32R)
                id_r = cpool.tile([128, 128], F32R)
                nc.vector.tensor_copy(hh_r[:], hh_sb[:])
                nc.vector.tensor_copy(id_r[:], id_sb[:])
            else:
                hh_r, id_r = hh_sb, id_sb

            for xt, ot in ((xre, ore), (xim, oim)):
                xv = xt[:].rearrange(
                    "(g p h) (i j) -> g h i p j", g=G, p=8, h=2, i=64, j=64
                )
                ov = ot[:].rearrange(
                    "(g c d h) (k l) -> g h k c d l", g=G, c=4, d=2, h=2, k=64, l=64
                )
                for g in range(G):
                    a = apool.tile([128, 512], F32)
                    for h in range(2):
                        eng = nc.sync if h == 0 else nc.scalar
                        eng.dma_start(
                            a[64 * h : 64 * h + 64].rearrange(
                                "q (p j) -> q p j", p=8, j=64
                            ),
                            xv[g, h],
                        )
                    ps1 = ps1pool.tile([128, 512], F32)
                    nc.tensor.matmul(
                        ps1[:], hh_sb[:], a[:], start=True, stop=True
                    )
                    b1 = b1pool.tile([128, 512], DT)
                    nc.vector.tensor_copy(b1[:], ps1[:])

                    psf1 = psf1pool.tile([128, 512], DT)
                    for c in range(4):
                        nc.tensor.transpose(
                            psf1[:, 128 * c : 128 * c + 128],
                            b1[:, 128 * c : 128 * c + 128],
                            id_r[:],
                        )
                    b2 = b2pool.tile([128, 512], DT)
                    nc.scalar.copy(b2[:], psf1[:])

                    ps2 = ps2pool.tile([128, 512], F32)
                    nc.tensor.matmul(
                        ps2[:], hh_r[:], b2[:], start=True, stop=True
                    )
                    b3 = b3pool.tile([128, 512], DT)
                    nc.vector.tensor_copy(b3[:], ps2[:])

                    psf2 = psf2pool.tile([128, 512], DT)
                    for c in range(4):
                        nc.tensor.transpose(
                            psf2[:, 128 * c : 128 * c + 128],
                            b3[:, 128 * c : 128 * c + 128],
                            id_r[:],
                        )
                    cc = cpool2.tile([128, 512], F32)
                    nc.scalar.copy(cc[:], psf2[:])

                    for h in range(2):
                        eng = nc.sync if h == 0 else nc.scalar
                        eng.dma_start(
                            ov[g, h],
                            cc[64 * h : 64 * h + 64].rearrange(
                                "q (c d l) -> q c d l", c=4, d=2, l=64
                            ),
                        )
    _split_excess_waits(nc)
    return nc


_NC_CACHE = {}


def _get_nc():
    key = (USE_F32R, BUFS)
    if key not in _NC_CACHE:
        _NC_CACHE[key] = _build()
    return _NC_CACHE[key]


def _run(x_re: np.ndarray, x_im: np.ndarray, trace: bool = False, tmpdir=None):
    nc = _get_nc()
    hh, ident = _host_constants()
    in_maps = []
    for b in range(NCORES):
        in_maps.append(
            {
                "x_re": np.ascontiguousarray(x_re[b]),
                "x_im": np.ascontiguousarray(x_im[b]),
                "hh": hh,
                "ident": ident,
            }
        )
    res = run_bass_kernel_spmd(
        nc, in_maps, list(range(NCORES)), trace=trace, tmpdir=tmpdir
    )
    return res


def kernel(x_re, x_im):
    x_re = np.asarray(x_re, dtype=np.float32)
    x_im = np.asarray(x_im, dtype=np.float32)
    res = _run(x_re, x_im, trace=False)
    out = np.empty((B, M, N), dtype=np.complex64)
    for b in range(NCORES):
        out.real[b] = res.results[b]["o_re"]
        out.imag[b] = res.results[b]["o_im"]
    return out



# revision 5
# speedup vs baseline: 2.3804x; 2.3804x over previous
"""Trainium2 Bass kernel for batched Hadamard transform.

Computes out = (x_re + i*x_im) @ H where H is the 4096x4096 Walsh-Hadamard
unitary (real, entries +-1/64).  Since H is real, out_re = x_re @ H and
out_im = x_im @ H independently.

Algorithm: H_4096 = H_64 (x) H_64 (Kronecker), so each 4096-row, viewed as a
64x64 matrix V, transforms as  H64 . V . H64  -- a 32x FLOP reduction vs the
dense matmul.  Implementation avoids PE transposes entirely by exploiting
out = lhsT^T @ rhs:

  stage 1 (contract i): lhsT = data chunk [128,128], rhs = HH (moving).
      Output is transposed for free: partitions become (p%2, j).
  stage 2 (contract j): lhsT = HH, rhs = full [128,512] stage-1 tile.

with HH = blockdiag(H64, H64) handling two 64-blocks per 128-partition op.
Everything runs in bf16 (H entries +-2^-6 are exact in bf16; tolerance is
2e-2), so matmuls stream at 1 cycle/row and DMA bytes are halved.  The host
pre-packs x into the exact SBUF tile layout so every DMA is contiguous, and
un-packs the (permuted) output tiles afterwards.

Sharding: data-parallel over the batch dim (8 batches -> 8 NeuronCores).
"""

import re
import numpy as np
import ml_dtypes

from concourse import bass, tile
import concourse.mybir as mybir
from concourse.bass_utils import run_bass_kernel_spmd
from concourse.tile import TileContext
from concourse.tile_sem_assignment import tick_to_sem


def _drain_and_barrier_split(self, tick_clock, wait_clock):
    # The stock kernel-tail drain carries one sem-wait per active proc on a
    # single instruction; this walrus build rejects >2 sync waits per
    # instruction ("Too many sync wait commands").  Emit one wait_ge per
    # proc instead, then a bare drain.
    gc = tick_clock.global_clock
    ticks = [int(v) for v in re.findall(r"\d+", repr(gc))]
    for proc, sem in sorted(self.sems.allocated().items()):
        if proc < len(ticks) and ticks[proc] > 0:
            self.nc.sync.wait_ge(sem, tick_to_sem(ticks[proc], proc))
    self.nc.sync.drain()
    self.nc.all_engine_barrier()
    assert self.sems is not None
    popped = self.nc._tile_sem_poison_stack.pop()
    assert popped is self._sem_poison
    self.nc.clear_and_free_semaphores(list(self.sems.allocated().values()))
    self.nc.all_engine_barrier()


TileContext._drain_and_barrier = _drain_and_barrier_split

_MAX_WAITS = 1


def _split_excess_waits(nc):
    """This walrus build rejects instructions with >2 sync-wait commands.
    Move excess waits onto same-engine NoOps inserted just before the
    instruction (engines execute their queue in order, so the sync semantics
    are preserved)."""
    n_split = 0
    for fn in nc.m.functions:
        for bb in fn.blocks:
            insts = list(bb.instructions)
            out = []
            for inst in insts:
                si = inst.sync_info
                waits = list(si.on_wait) if si and si.on_wait else []
                if len(waits) > _MAX_WAITS:
                    extra = waits[: len(waits) - _MAX_WAITS]
                    keep = waits[len(waits) - _MAX_WAITS :]
                    for ci in range(0, len(extra), _MAX_WAITS):
                        chunk = extra[ci : ci + _MAX_WAITS]
                        n_split += 1
                        nop = mybir.InstNoOp(
                            name=f"waitnop-{n_split}-{inst.name}",
                            engine=inst.engine,
                            sync_info=mybir.SyncInfo(
                                on_wait=list(chunk), on_update=[]
                            ),
                        )
                        out.append(nop)
                    inst.sync_info = mybir.SyncInfo(
                        on_wait=list(keep), on_update=list(si.on_update)
                    )
                out.append(inst)
            if len(out) != len(insts):
                bb.instructions = out
    return n_split


B, M, N = 8, 512, 4096
NCORES = 8
G = 32           # row-groups per tensor; 16 rows per group
AB = 4           # groups per DMA tile
NT = 2 * (G // AB)   # DMA tiles: re + im
F32 = mybir.dt.float32
BF16 = mybir.dt.bfloat16
NPBF16 = ml_dtypes.bfloat16


def _hadamard(n: int) -> np.ndarray:
    h = np.array([[1.0]], dtype=np.float64)
    while h.shape[0] < n:
        h = np.block([[h, h], [h, -h]])
    return h


def _host_hh() -> np.ndarray:
    h64 = (_hadamard(64) / 8.0).astype(NPBF16)  # +-2^-6: exact in bf16
    hh = np.zeros((128, 128), dtype=NPBF16)
    hh[:64, :64] = h64
    hh[64:, 64:] = h64
    return hh


def _pack(x: np.ndarray) -> np.ndarray:
    """[512, 4096] f32 row-major -> [8, 128, 4*512] bf16 SBUF tile layout.

    Row r = ((gg*4 + ga)*8 + p)*2 + h, col = i*64 + j maps to
    X[gg, h*64 + i, ga*512 + p*64 + j]."""
    v = x.astype(NPBF16).reshape(8, 4, 8, 2, 64, 64)
    return np.ascontiguousarray(v.transpose(0, 3, 4, 1, 2, 5)).reshape(8, 128, 2048)


def _unpack(o: np.ndarray) -> np.ndarray:
    """[8, 128, 2048] bf16 output tiles -> [512, 4096] f32.

    O[gg, d*64 + l, ga*512 + c*128 + h*64 + k] is the output element at
    row gg*64 + ga*16 + c*4 + d*2 + h, col k*64 + l."""
    v = o.reshape(8, 2, 64, 4, 4, 2, 64)
    return (
        np.ascontiguousarray(v.transpose(0, 3, 4, 1, 5, 6, 2))
        .reshape(512, 4096)
        .astype(np.float32)
    )


def _build():
    nc = bass.Bass()
    xin = nc.dram_tensor("xin", [NT, 128, AB * 512], BF16, kind="ExternalInput")
    hh = nc.dram_tensor("hh", [128, 128], BF16, kind="ExternalInput")
    oout = nc.dram_tensor("oout", [NT, 128, AB * 512], BF16, kind="ExternalOutput")

    with tile.TileContext(nc) as tc:
        with (
            tc.tile_pool(name="const", bufs=1) as cpool,
            tc.tile_pool(name="a", bufs=3) as apool,
            tc.tile_pool(name="b", bufs=4) as bpool,
            tc.tile_pool(name="cc", bufs=3) as ccpool,
            tc.tile_pool(name="ps1", bufs=3, space="PSUM") as ps1pool,
            tc.tile_pool(name="ps2", bufs=3, space="PSUM") as ps2pool,
        ):
            hh_sb = cpool.tile([128, 128], BF16)
            nc.sync.dma_start(hh_sb[:], hh[:])

            # round-robin the 8 PSUM->SBUF copies per tile across engines
            def copy_to(eng, out, in_):
                if eng is nc.scalar:
                    eng.copy(out, in_)
                else:
                    eng.tensor_copy(out, in_)

            # gpsimd cannot read PSUM on this target; split DVE/ACT evenly
            copy_engines = [
                nc.vector, nc.scalar, nc.scalar, nc.vector,
                nc.vector, nc.scalar, nc.scalar, nc.vector,
            ]

            for t in range(NT):
                a = apool.tile([128, AB * 512], BF16)
                nc.sync.dma_start(a[:], xin[t])
                cc = ccpool.tile([128, AB * 512], BF16)
                for ga in range(AB):
                    ps1 = ps1pool.tile([128, 512], F32)
                    for c in range(4):
                        lo = 512 * ga + 128 * c
                        nc.tensor.matmul(
                            ps1[:, 128 * c : 128 * c + 128],
                            a[:, lo : lo + 128],
                            hh_sb[:],
                            start=True,
                            stop=True,
                        )
                    b = bpool.tile([128, 512], BF16)
                    copy_to(copy_engines[2 * ga], b[:], ps1[:])
                    ps2 = ps2pool.tile([128, 512], F32)
                    nc.tensor.matmul(ps2[:], hh_sb[:], b[:], start=True, stop=True)
                    copy_to(
                        copy_engines[2 * ga + 1],
                        cc[:, 512 * ga : 512 * ga + 512],
                        ps2[:],
                    )
                nc.scalar.dma_start(oout[t], cc[:])
    _split_excess_waits(nc)
    return nc


_NC_CACHE = {}


def _get_nc():
    key = (AB,)
    if key not in _NC_CACHE:
        _NC_CACHE[key] = _build()
    return _NC_CACHE[key]


def _run(x_re: np.ndarray, x_im: np.ndarray, trace: bool = False, tmpdir=None):
    nc = _get_nc()
    hh = _host_hh()
    in_maps = []
    for b in range(NCORES):
        xp = np.concatenate([_pack(x_re[b]), _pack(x_im[b])], axis=0)
        in_maps.append({"xin": xp, "hh": hh})
    res = run_bass_kernel_spmd(
        nc, in_maps, list(range(NCORES)), trace=trace, tmpdir=tmpdir
    )
    return res


def kernel(x_re, x_im):
    x_re = np.asarray(x_re, dtype=np.float32)
    x_im = np.asarray(x_im, dtype=np.float32)
    res = _run(x_re, x_im, trace=False)
    out = np.empty((B, M, N), dtype=np.complex64)
    for b in range(NCORES):
        o = res.results[b]["oout"]
        out.real[b] = _unpack(o[: NT // 2])
        out.imag[b] = _unpack(o[NT // 2 :])
    return out


# revision 9
# speedup vs baseline: 2.6470x; 1.1120x over previous
"""Trainium2 Bass kernel for batched Hadamard transform.

Computes out = (x_re + i*x_im) @ H where H is the 4096x4096 Walsh-Hadamard
unitary (real, entries +-1/64).  Since H is real, out_re = x_re @ H and
out_im = x_im @ H independently.

Algorithm: H_4096 = H_64 (x) H_64 (Kronecker), so each 4096-row, viewed as a
64x64 matrix V, transforms as  H64 . V . H64  -- a 32x FLOP reduction vs the
dense matmul.  Implementation avoids PE transposes entirely by exploiting
out = lhsT^T @ rhs:

  stage 1 (contract i): lhsT = data chunk [128,128], rhs = HH (moving).
      Output is transposed for free: partitions become (p%2, j).
  stage 2 (contract j): lhsT = HH, rhs = full [128,512] stage-1 tile.

with HH = blockdiag(H64, H64) handling two 64-blocks per 128-partition op.
Everything runs in bf16 (H entries +-2^-6 are exact in bf16; tolerance is
2e-2), so matmuls stream at 1 cycle/row and DMA bytes are halved.  The host
pre-packs x into the exact SBUF tile layout so every DMA is contiguous, and
un-packs the (permuted) output tiles afterwards.

Sharding: data-parallel over the batch dim (8 batches -> 8 NeuronCores).
"""

import re
import numpy as np
import ml_dtypes

from concourse import bass, tile
import concourse.mybir as mybir
from concourse.bass_utils import run_bass_kernel_spmd
from concourse.tile import TileContext
from concourse.tile_sem_assignment import tick_to_sem


def _drain_and_barrier_split(self, tick_clock, wait_clock):
    # The stock kernel-tail drain carries one sem-wait per active proc on a
    # single instruction; this walrus build rejects >2 sync waits per
    # instruction ("Too many sync wait commands").  Emit one wait_ge per
    # proc instead, then a bare drain.
    gc = tick_clock.global_clock
    ticks = [int(v) for v in re.findall(r"\d+", repr(gc))]
    for proc, sem in sorted(self.sems.allocated().items()):
        if proc < len(ticks) and ticks[proc] > 0:
            self.nc.sync.wait_ge(sem, tick_to_sem(ticks[proc], proc))
    self.nc.sync.drain()
    self.nc.all_engine_barrier()
    assert self.sems is not None
    popped = self.nc._tile_sem_poison_stack.pop()
    assert popped is self._sem_poison
    self.nc.clear_and_free_semaphores(list(self.sems.allocated().values()))
    self.nc.all_engine_barrier()


TileContext._drain_and_barrier = _drain_and_barrier_split

_MAX_WAITS = 1


def _split_excess_waits(nc):
    """This walrus build rejects instructions with >2 sync-wait commands.
    Move excess waits onto same-engine NoOps inserted just before the
    instruction (engines execute their queue in order, so the sync semantics
    are preserved)."""
    n_split = 0
    for fn in nc.m.functions:
        for bb in fn.blocks:
            insts = list(bb.instructions)
            out = []
            for inst in insts:
                si = inst.sync_info
                waits = list(si.on_wait) if si and si.on_wait else []
                if len(waits) > _MAX_WAITS:
                    extra = waits[: len(waits) - _MAX_WAITS]
                    keep = waits[len(waits) - _MAX_WAITS :]
                    for ci in range(0, len(extra), _MAX_WAITS):
                        chunk = extra[ci : ci + _MAX_WAITS]
                        n_split += 1
                        nop = mybir.InstNoOp(
                            name=f"waitnop-{n_split}-{inst.name}",
                            engine=inst.engine,
                            sync_info=mybir.SyncInfo(
                                on_wait=list(chunk), on_update=[]
                            ),
                        )
                        out.append(nop)
                    inst.sync_info = mybir.SyncInfo(
                        on_wait=list(keep), on_update=list(si.on_update)
                    )
                out.append(inst)
            if len(out) != len(insts):
                bb.instructions = out
    return n_split


import os

B, M, N = 8, 512, 4096
NCORES = 8
G = 32           # row-groups per tensor; 16 rows per group
AB = int(os.environ.get("HAD_AB", "8"))       # groups per DMA tile
CM = int(os.environ.get("HAD_CM", "2"))       # groups per PSUM tile / copy
ODMA = os.environ.get("HAD_ODMA", "sync")     # engine issuing out-DMAs
NT = 2 * (G // AB)   # DMA tiles: re + im
F32 = mybir.dt.float32
BF16 = mybir.dt.bfloat16
NPBF16 = ml_dtypes.bfloat16


def _hadamard(n: int) -> np.ndarray:
    h = np.array([[1.0]], dtype=np.float64)
    while h.shape[0] < n:
        h = np.block([[h, h], [h, -h]])
    return h


def _host_hh() -> np.ndarray:
    h64 = (_hadamard(64) / 8.0).astype(NPBF16)  # +-2^-6: exact in bf16
    hh = np.zeros((128, 128), dtype=NPBF16)
    hh[:64, :64] = h64
    hh[64:, 64:] = h64
    return hh


def _pack(x: np.ndarray) -> np.ndarray:
    """[512, 4096] f32 row-major -> [G//AB, 128, AB*512] bf16 SBUF tiles.

    Row r = ((gg*AB + ga)*8 + p)*2 + h, col = i*64 + j maps to
    X[gg, h*64 + i, ga*512 + p*64 + j]."""
    gg = G // AB
    v = x.astype(NPBF16).reshape(gg, AB, 8, 2, 64, 64)
    return np.ascontiguousarray(v.transpose(0, 3, 4, 1, 2, 5)).reshape(
        gg, 128, AB * 512
    )


def _unpack(o: np.ndarray) -> np.ndarray:
    """[G//AB, 128, AB*512] bf16 output tiles -> [512, 4096] f32.

    O[gg, d*64 + l, ga*512 + c*128 + h*64 + k] is the output element at
    row (gg*AB + ga)*16 + c*4 + d*2 + h, col k*64 + l."""
    gg = G // AB
    v = o.reshape(gg, 2, 64, AB, 4, 2, 64)
    return (
        np.ascontiguousarray(v.transpose(0, 3, 4, 1, 5, 6, 2))
        .reshape(512, 4096)
        .astype(np.float32)
    )


def _build():
    nc = bass.Bass()
    xin = nc.dram_tensor("xin", [NT, 128, AB * 512], BF16, kind="ExternalInput")
    hh = nc.dram_tensor("hh", [128, 128], BF16, kind="ExternalInput")
    oout = nc.dram_tensor("oout", [NT, 128, AB * 512], BF16, kind="ExternalOutput")

    with tile.TileContext(nc) as tc:
        with (
            tc.tile_pool(name="const", bufs=1) as cpool,
            tc.tile_pool(name="a", bufs=2) as apool,
            tc.tile_pool(name="b", bufs=3) as bpool,
            tc.tile_pool(name="cc", bufs=2) as ccpool,
            tc.tile_pool(name="ps1", bufs=2, space="PSUM") as ps1pool,
            tc.tile_pool(name="ps2", bufs=2, space="PSUM") as ps2pool,
        ):
            hh_sb = cpool.tile([128, 128], BF16)
            nc.sync.dma_start(hh_sb[:], hh[:])

            # gpsimd cannot read PSUM on this target; split DVE/ACT evenly
            def copy_to(eng, out, in_):
                if eng is nc.scalar:
                    eng.copy(out, in_)
                else:
                    eng.tensor_copy(out, in_)

            odma = nc.sync if ODMA == "sync" else nc.scalar

            for t in range(NT):
                a = apool.tile([128, AB * 512], BF16)
                nc.sync.dma_start(a[:], xin[t])
                cc = ccpool.tile([128, AB * 512], BF16)
                for gp in range(AB // CM):
                    ps1 = ps1pool.tile([128, CM * 512], F32)
                    for gi in range(CM):
                        for c in range(4):
                            lo = 512 * (gp * CM + gi) + 128 * c
                            nc.tensor.matmul(
                                ps1[:, 512 * gi + 128 * c : 512 * gi + 128 * c + 128],
                                a[:, lo : lo + 128],
                                hh_sb[:],
                                start=True,
                                stop=True,
                            )
                    b = bpool.tile([128, CM * 512], BF16)
                    copy_to(nc.vector if gp % 2 == 0 else nc.scalar, b[:], ps1[:])
                    ps2 = ps2pool.tile([128, CM * 512], F32)
                    for gi in range(CM):
                        # back-to-back stage-2 matmuls share the hh stationary
                        nc.tensor.matmul(
                            ps2[:, 512 * gi : 512 * gi + 512],
                            hh_sb[:],
                            b[:, 512 * gi : 512 * gi + 512],
                            start=True,
                            stop=True,
                        )
                    copy_to(
                        nc.scalar if gp % 2 == 0 else nc.vector,
                        cc[:, 512 * gp * CM : 512 * (gp * CM + CM)],
                        ps2[:],
                    )
                odma.dma_start(oout[t], cc[:])
    _split_excess_waits(nc)
    return nc


_NC_CACHE = {}


def _get_nc():
    key = (AB, CM, ODMA)
    if key not in _NC_CACHE:
        _NC_CACHE[key] = _build()
    return _NC_CACHE[key]


def _run(x_re: np.ndarray, x_im: np.ndarray, trace: bool = False, tmpdir=None):
    nc = _get_nc()
    hh = _host_hh()
    in_maps = []
    for b in range(NCORES):
        xp = np.concatenate([_pack(x_re[b]), _pack(x_im[b])], axis=0)
        in_maps.append({"xin": xp, "hh": hh})
    res = run_bass_kernel_spmd(
        nc, in_maps, list(range(NCORES)), trace=trace, tmpdir=tmpdir
    )
    return res


def kernel(x_re, x_im):
    x_re = np.asarray(x_re, dtype=np.float32)
    x_im = np.asarray(x_im, dtype=np.float32)
    res = _run(x_re, x_im, trace=False)
    out = np.empty((B, M, N), dtype=np.complex64)
    for b in range(NCORES):
        o = res.results[b]["oout"]
        out.real[b] = _unpack(o[: NT // 2])
        out.imag[b] = _unpack(o[NT // 2 :])
    return out


# revision 14
# speedup vs baseline: 2.9273x; 1.1059x over previous
"""Trainium2 Bass kernel for batched Hadamard transform.

Computes out = (x_re + i*x_im) @ H where H is the 4096x4096 Walsh-Hadamard
unitary (real, entries +-1/64).  Since H is real, out_re = x_re @ H and
out_im = x_im @ H independently.

Algorithm: H_4096 = H_64 (x) H_64 (Kronecker), so each 4096-row, viewed as a
64x64 matrix V, transforms as  H64 . V . H64  -- a 32x FLOP reduction vs the
dense matmul.  Implementation avoids PE transposes entirely by exploiting
out = lhsT^T @ rhs:

  stage 1 (contract i): lhsT = data chunk [128,128], rhs = HH (moving).
      Output is transposed for free: partitions become (p%2, j).
  stage 2 (contract j): lhsT = HH, rhs = full [128,512] stage-1 tile.

with HH = blockdiag(H64, H64) handling two 64-blocks per 128-partition op.
Everything runs in bf16 (H entries +-2^-6 are exact in bf16; tolerance is
2e-2), so matmuls stream at 1 cycle/row and DMA bytes are halved.  The host
pre-packs x into the exact SBUF tile layout so every DMA is contiguous, and
un-packs the (permuted) output tiles afterwards.

Sharding: data-parallel over the batch dim (8 batches -> 8 NeuronCores).
"""

import re
import numpy as np
import ml_dtypes

from concourse import bass, tile
import concourse.mybir as mybir
from concourse.bass_utils import run_bass_kernel_spmd
from concourse.tile import TileContext
from concourse.tile_sem_assignment import tick_to_sem


def _drain_and_barrier_split(self, tick_clock, wait_clock):
    # The stock kernel-tail drain carries one sem-wait per active proc on a
    # single instruction; this walrus build rejects >2 sync waits per
    # instruction ("Too many sync wait commands").  Emit one wait_ge per
    # proc instead, then a bare drain.
    gc = tick_clock.global_clock
    ticks = [int(v) for v in re.findall(r"\d+", repr(gc))]
    for proc, sem in sorted(self.sems.allocated().items()):
        if proc < len(ticks) and ticks[proc] > 0:
            self.nc.sync.wait_ge(sem, tick_to_sem(ticks[proc], proc))
    self.nc.sync.drain()
    self.nc.all_engine_barrier()
    assert self.sems is not None
    popped = self.nc._tile_sem_poison_stack.pop()
    assert popped is self._sem_poison
    self.nc.clear_and_free_semaphores(list(self.sems.allocated().values()))
    self.nc.all_engine_barrier()


TileContext._drain_and_barrier = _drain_and_barrier_split

_MAX_WAITS = 1


def _split_excess_waits(nc):
    """This walrus build rejects instructions with >2 sync-wait commands.
    Move excess waits onto same-engine NoOps inserted just before the
    instruction (engines execute their queue in order, so the sync semantics
    are preserved)."""
    n_split = 0
    for fn in nc.m.functions:
        for bb in fn.blocks:
            insts = list(bb.instructions)
            out = []
            for inst in insts:
                si = inst.sync_info
                waits = list(si.on_wait) if si and si.on_wait else []
                if len(waits) > _MAX_WAITS:
                    extra = waits[: len(waits) - _MAX_WAITS]
                    keep = waits[len(waits) - _MAX_WAITS :]
                    for ci in range(0, len(extra), _MAX_WAITS):
                        chunk = extra[ci : ci + _MAX_WAITS]
                        n_split += 1
                        nop = mybir.InstNoOp(
                            name=f"waitnop-{n_split}-{inst.name}",
                            engine=inst.engine,
                            sync_info=mybir.SyncInfo(
                                on_wait=list(chunk), on_update=[]
                            ),
                        )
                        out.append(nop)
                    inst.sync_info = mybir.SyncInfo(
                        on_wait=list(keep), on_update=list(si.on_update)
                    )
                out.append(inst)
            if len(out) != len(insts):
                bb.instructions = out
    return n_split


import os

B, M, N = 8, 512, 4096
NCORES = 8
G = 32           # row-groups per tensor; 16 rows per group
AB = int(os.environ.get("HAD_AB", "8"))       # groups per DMA tile
CM = int(os.environ.get("HAD_CM", "2"))       # groups per PSUM tile / copy
ODMA = os.environ.get("HAD_ODMA", "sync")     # engine issuing out-DMAs
LOOKAHEAD = int(os.environ.get("HAD_LA", "1"))  # stage-2 emission delay
NT = 2 * (G // AB)   # DMA tiles: re + im
F32 = mybir.dt.float32
BF16 = mybir.dt.bfloat16
NPBF16 = ml_dtypes.bfloat16


def _hadamard(n: int) -> np.ndarray:
    h = np.array([[1.0]], dtype=np.float64)
    while h.shape[0] < n:
        h = np.block([[h, h], [h, -h]])
    return h


def _host_hh() -> np.ndarray:
    h64 = (_hadamard(64) / 8.0).astype(NPBF16)  # +-2^-6: exact in bf16
    hh = np.zeros((128, 128), dtype=NPBF16)
    hh[:64, :64] = h64
    hh[64:, 64:] = h64
    return hh


def _pack(x: np.ndarray) -> np.ndarray:
    """[512, 4096] f32 row-major -> [G//AB, 128, AB*512] bf16 SBUF tiles.

    Row r = ((gg*AB + ga)*8 + p)*2 + h, col = i*64 + j maps to
    X[gg, h*64 + i, ga*512 + p*64 + j]."""
    gg = G // AB
    v = x.astype(NPBF16).reshape(gg, AB, 8, 2, 64, 64)
    return np.ascontiguousarray(v.transpose(0, 3, 4, 1, 2, 5)).reshape(
        gg, 128, AB * 512
    )


def _unpack(o: np.ndarray) -> np.ndarray:
    """[G//AB, 128, AB*512] bf16 output tiles -> [512, 4096] f32.

    O[gg, d*64 + l, ga*512 + c*128 + h*64 + k] is the output element at
    row (gg*AB + ga)*16 + c*4 + d*2 + h, col k*64 + l."""
    gg = G // AB
    v = o.reshape(gg, 2, 64, AB, 4, 2, 64)
    return (
        np.ascontiguousarray(v.transpose(0, 3, 4, 1, 5, 6, 2))
        .reshape(512, 4096)
        .astype(np.float32)
    )


def _build():
    nc = bass.Bass()
    xin = nc.dram_tensor("xin", [NT, 128, AB * 512], BF16, kind="ExternalInput")
    hh = nc.dram_tensor("hh", [128, 128], BF16, kind="ExternalInput")
    oout = nc.dram_tensor("oout", [NT, 128, AB * 512], BF16, kind="ExternalOutput")

    with tile.TileContext(nc) as tc:
        with (
            tc.tile_pool(name="const", bufs=1) as cpool,
            tc.tile_pool(name="a", bufs=2) as apool,
            tc.tile_pool(name="b", bufs=3) as bpool,
            tc.tile_pool(name="cc", bufs=2) as ccpool,
            tc.tile_pool(name="ps1", bufs=2, space="PSUM") as ps1pool,
            tc.tile_pool(name="ps2", bufs=2, space="PSUM") as ps2pool,
        ):
            hh_sb = cpool.tile([128, 128], BF16)
            nc.sync.dma_start(hh_sb[:], hh[:])

            # gpsimd cannot read PSUM on this target; split DVE/ACT evenly
            def copy_to(eng, out, in_):
                if eng is nc.scalar:
                    eng.copy(out, in_)
                else:
                    eng.tensor_copy(out, in_)

            odma = nc.sync if ODMA == "sync" else nc.scalar
            GP = AB // CM

            # Software-pipelined emission: engines execute their queues in
            # emission order, so stage-2 of group-pair k is emitted AFTER
            # stage-1 of k+LOOKAHEAD.  Otherwise the PE sits in-queue behind
            # a matmul that waits on the DVE/ACT copy of the previous group.
            sched = [(t, gp) for t in range(NT) for gp in range(GP)]
            a_tiles, cc_tiles, st1 = {}, {}, {}

            def stage1(k):
                t, gp = sched[k]
                if gp == 0:
                    a = apool.tile([128, AB * 512], BF16, name="a")
                    nc.sync.dma_start(a[:], xin[t])
                    a_tiles[t] = a
                    cc_tiles[t] = ccpool.tile([128, AB * 512], BF16, name="cc")
                a = a_tiles[t]
                ps1 = ps1pool.tile([128, CM * 512], F32)
                for gi in range(CM):
                    for c in range(4):
                        lo = 512 * (gp * CM + gi) + 128 * c
                        nc.tensor.matmul(
                            ps1[:, 512 * gi + 128 * c : 512 * gi + 128 * c + 128],
                            a[:, lo : lo + 128],
                            hh_sb[:],
                            start=True,
                            stop=True,
                        )
                b = bpool.tile([128, CM * 512], BF16)
                copy_to(nc.vector if k % 2 == 0 else nc.scalar, b[:], ps1[:])
                st1[k] = b

            def stage2(k):
                t, gp = sched[k]
                b = st1.pop(k)
                ps2 = ps2pool.tile([128, CM * 512], F32)
                for gi in range(CM):
                    # back-to-back stage-2 matmuls share the hh stationary
                    nc.tensor.matmul(
                        ps2[:, 512 * gi : 512 * gi + 512],
                        hh_sb[:],
                        b[:, 512 * gi : 512 * gi + 512],
                        start=True,
                        stop=True,
                    )
                copy_to(
                    nc.scalar if k % 2 == 0 else nc.vector,
                    cc_tiles[t][:, 512 * gp * CM : 512 * (gp * CM + CM)],
                    ps2[:],
                )
                if gp == GP - 1:
                    odma.dma_start(oout[t], cc_tiles[t][:])

            for k in range(len(sched) + LOOKAHEAD):
                if k < len(sched):
                    stage1(k)
                if k >= LOOKAHEAD:
                    stage2(k - LOOKAHEAD)
    _split_excess_waits(nc)
    return nc


_NC_CACHE = {}


def _get_nc():
    key = (AB, CM, ODMA, LOOKAHEAD)
    if key not in _NC_CACHE:
        _NC_CACHE[key] = _build()
    return _NC_CACHE[key]


def _run(x_re: np.ndarray, x_im: np.ndarray, trace: bool = False, tmpdir=None):
    nc = _get_nc()
    hh = _host_hh()
    in_maps = []
    for b in range(NCORES):
        xp = np.concatenate([_pack(x_re[b]), _pack(x_im[b])], axis=0)
        in_maps.append({"xin": xp, "hh": hh})
    res = run_bass_kernel_spmd(
        nc, in_maps, list(range(NCORES)), trace=trace, tmpdir=tmpdir
    )
    return res


def kernel(x_re, x_im):
    x_re = np.asarray(x_re, dtype=np.float32)
    x_im = np.asarray(x_im, dtype=np.float32)
    res = _run(x_re, x_im, trace=False)
    out = np.empty((B, M, N), dtype=np.complex64)
    for b in range(NCORES):
        o = res.results[b]["oout"]
        out.real[b] = _unpack(o[: NT // 2])
        out.imag[b] = _unpack(o[NT // 2 :])
    return out
